# revision 17
# baseline (speedup 1.0000x reference)
"""Trainium2 Bass kernel for nn_MultiHeadAttention_79224966742350.

Full (unsharded) inputs in, full output out. Internally: 8-way SPMD over
8 NeuronCores, sharded batch x head-group: core c handles batch c//4 and
heads [4*(c%4), 4*(c%4)+4) (=256 of the 1024 projection dims). Each core
computes its partial x @ wo_cols contribution; the host sums the 4
partials per batch and adds the (adjusted) output bias.

Key design:
  - The HOST pre-transposes and pre-casts everything the device would
    otherwise have to transpose on the PE: x^T [D, S] and w^T slices are
    shipped bf16, eliminating all PE transpose instructions.
  - All matmuls run in bf16 (1 cycle/row at any moving size on TRN2, no
    fp32r small-N penalty); PSUM accumulation stays fp32.
  - k-bias is dropped exactly (softmax shift invariance); v-bias is
    folded into the host-side output bias (bo' = bo + wo @ bv); q-bias
    is a K=1 ones matmul.
  - The next chunk's Q-projection is issued between attention and the
    output projection so the PE stays busy while the last head's
    normalization chain (DVE reciprocal + Pool broadcast) completes.
  - Engine assignment: Act = exp + Q/K evictions, DVE = V evictions /
    triangle mask / normalization / output evictions, Pool(GpSimd) =
    partition broadcast; PE does only real matmuls.

Device-side per core (S=2048 tokens, D=1024, 4 heads x d_k=64), per
512-token chunk j: project Q^T/K^T [head_dim, tok] and V [tok, head_dim]
from pre-transposed inputs, then attention for q-chunk j with transposed
scores S^T = K^T.T @ Q^T -> [k_tok, q_tok]: softmax numerator via ACT Exp
from PSUM restricted to the causally-live range, diagonal-band triangle
masking via DVE multiply, denominator via a ones column appended to V
(attnV matmul M=65), normalization via DVE reciprocal + Pool
partition_broadcast + DVE multiply, then the output projection.
"""

import sys

sys.path.insert(0, "/opt/trn_rl_repo")

import numpy as np
import ml_dtypes

import concourse.bacc as bacc
import concourse.mybir as mybir
import concourse.tile as tile
from concourse.bass_utils import run_bass_kernel_spmd

F32 = mybir.dt.float32
BF16 = mybir.dt.bfloat16
AF = mybir.ActivationFunctionType
NPBF = ml_dtypes.bfloat16

B = 2
S = 2048
D = 1024
DK = 64
HPC = 4          # heads per core
HD = HPC * DK    # 256 projection dims per core
NCORES = 8
NJ = S // 512    # 512-token chunks
P = 128


def build_nc():
    nc = bacc.Bacc("TRN2", target_bir_lowering=False, debug=False,
                   num_devices=NCORES)

    xqT = nc.dram_tensor("xqT", [D, S], BF16, kind="ExternalInput").ap()
    xkT = nc.dram_tensor("xkT", [D, S], BF16, kind="ExternalInput").ap()
    xvT = nc.dram_tensor("xvT", [D, S], BF16, kind="ExternalInput").ap()
    wqT = nc.dram_tensor("wqT", [D, HD], BF16, kind="ExternalInput").ap()
    wkT = nc.dram_tensor("wkT", [D, HD], BF16, kind="ExternalInput").ap()
    wvT = nc.dram_tensor("wvT", [D, HD], BF16, kind="ExternalInput").ap()
    woT = nc.dram_tensor("woT", [HD, D], BF16, kind="ExternalInput").ap()
    bq = nc.dram_tensor("bq", [1, HD], BF16, kind="ExternalInput").ap()
    out = nc.dram_tensor("out", [S, D], BF16, kind="ExternalOutput").ap()

    with tile.TileContext(nc) as tc:
        with (
            tc.tile_pool(name="const", bufs=1) as const,
            tc.tile_pool(name="wtp", bufs=1) as wtp,
            tc.tile_pool(name="qkv", bufs=1) as qkv,
            tc.tile_pool(name="opool", bufs=2) as opool,
            tc.tile_pool(name="xtp", bufs=2) as xtp,
            tc.tile_pool(name="ppool", bufs=6) as ppool,
            tc.tile_pool(name="small", bufs=2) as small,
            tc.tile_pool(name="outsb", bufs=3) as outsbp,
            tc.tile_pool(name="psA", bufs=2, space="PSUM") as psA,
            tc.tile_pool(name="psS", bufs=4, space="PSUM") as psS,
            tc.tile_pool(name="psO", bufs=2, space="PSUM") as psO,
        ):
            ones = const.tile([1, 512], BF16, tag="ones")
            nc.gpsimd.memset(ones[:], 1.0)
            # triangle mask: tri[p, f] = 1.0 where f >= p else 0.0
            tri = const.tile([P, P], BF16, tag="tri")
            nc.gpsimd.memset(tri[:], 1.0)
            nc.gpsimd.affine_select(
                out=tri[:], in_=tri[:],
                compare_op=mybir.AluOpType.is_ge, fill=0.0,
                base=0, pattern=[[1, P]], channel_multiplier=-1)

            # exp-table preload: a 1-elem Exp so att0's first real exp
            # doesn't pay the ACT_TABLE_LOAD
            dummy = const.tile([1, 1], F32, tag="dummy")
            nc.scalar.activation(dummy[:], dummy[:], AF.Exp)

            # ---- weights: DMA pre-transposed bf16 straight into SBUF ----
            # issue order tracks first use: wq/xq gate the first matmul,
            # wo_sb isn't needed until wo0
            wq_sb = wtp.tile([P, 8, HD], BF16, tag="wq")
            wk_sb = wtp.tile([P, 8, HD], BF16, tag="wk")
            wv_sb = wtp.tile([P, 8, HD], BF16, tag="wv")
            wo_sb = wtp.tile([P, 2, D], BF16, tag="wo")
            bq_sb = wtp.tile([1, HD], BF16, tag="bq")
            wqr = wqT.rearrange("(d p) c -> p d c", p=P)
            for dd in range(8):
                nc.sync.dma_start(out=wq_sb[:, dd, :], in_=wqr[:, dd, :])
            nc.sync.dma_start(out=bq_sb[:], in_=bq)

            # ---- storage for Q^T, K^T (transposed) and V (natural) ------
            Qt = [qkv.tile([P, S], BF16, tag=f"Q{m}", name=f"Q{m}")
                  for m in range(2)]
            Kt = [qkv.tile([P, S], BF16, tag=f"K{m}", name=f"K{m}")
                  for m in range(2)]
            # V with a ones column per head: [tok_tile, tok, head, 65]
            V = qkv.tile([P, 16, HPC, DK + 1], BF16, tag="V")
            nc.gpsimd.memset(V[:, :, :, DK:DK + 1], 1.0)

            # ---- projections for one 512-token chunk --------------------
            def proj_dma(j):
                xts = {}
                for name, x_ap in (("q", xqT), ("k", xkT), ("v", xvT)):
                    xT = xtp.tile([P, 8, 512], BF16, tag=f"xT{name}",
                                  name=f"xT{name}")
                    xr = x_ap.rearrange("(d p) s -> p d s", p=P)
                    if j == 0:
                        # d-sliced so the first matmuls gate on ~192KB
                        # instead of the full 1.5MB
                        for dd in range(8):
                            nc.sync.dma_start(
                                out=xT[:, dd, :],
                                in_=xr[:, dd, j * 512:(j + 1) * 512])
                    else:
                        nc.sync.dma_start(
                            out=xT[:], in_=xr[:, :, j * 512:(j + 1) * 512])
                    xts[name] = xT
                    if j == 0 and name == "q":
                        nc.sync.dma_start(
                            out=wk_sb[:],
                            in_=wkT.rearrange("(d p) c -> p d c", p=P))
                    elif j == 0 and name == "k":
                        nc.sync.dma_start(
                            out=wv_sb[:],
                            in_=wvT.rearrange("(d p) c -> p d c", p=P))
                    elif j == 0 and name == "v":
                        nc.sync.dma_start(
                            out=wo_sb[:],
                            in_=woT.rearrange("(m p) c -> p m c", p=P))
                return xts

            def proj_qk(j, xts, which):
                # Q^T or K^T: [head_dim 256, tok 512] for this chunk
                with nc.named_scope(f"proj{j}{which}"):
                    xT, wsb, b_sb, dst = {
                        "q": (xts["q"], wq_sb, bq_sb, Qt),
                        "k": (xts["k"], wk_sb, None, Kt),
                    }[which]
                    for m in range(2):
                        ps = psA.tile([P, 512], F32, tag="ps", name="ps")
                        for d in range(8):
                            nc.tensor.matmul(
                                ps[:], wsb[:, d, m * P:(m + 1) * P],
                                xT[:, d, :], start=(d == 0),
                                stop=(d == 7 and b_sb is None))
                        if b_sb is not None:
                            nc.tensor.matmul(
                                ps[:], b_sb[0:1, m * P:(m + 1) * P],
                                ones[:], start=False, stop=True)
                        nc.scalar.activation(
                            dst[m][:, j * 512:(j + 1) * 512], ps[:], AF.Copy)

            def proj_v(j, xts):
                # V natural: [tok 128, head_dim 256] per token tile
                with nc.named_scope(f"proj{j}v"):
                    for t in range(4):
                        ps = psA.tile([P, 512], F32, tag="ps", name="ps")
                        for d in range(8):
                            nc.tensor.matmul(
                                ps[:, 0:HD], xts["v"][:, d, t * P:(t + 1) * P],
                                wv_sb[:, d, :], start=(d == 0), stop=(d == 7))
                        nc.vector.tensor_copy(
                            V[:, j * 4 + t, :, 0:DK],
                            ps[:, 0:HD].rearrange("p (h c) -> p h c", c=DK))

            # ---- attention + output projection for one 512-q chunk ------
            def att_heads(jq):
                Ot = [opool.tile([P, 512], BF16, tag=f"O{m}", name=f"O{m}")
                      for m in range(2)]
                with nc.named_scope(f"att{jq}"):
                    for h in range(HPC):
                        m, off = h // 2, DK * (h % 2)
                        nk = 4 * (jq + 1)
                        po = psO.tile([DK + 1, 512], F32)

                        def col0(i):
                            # first causally-live column of k-chunk i's tile
                            return max(0, 128 * i - 512 * jq)

                        def score(i):
                            c0 = col0(i)
                            ps = psS.tile([P, 512], F32)
                            nc.tensor.matmul(
                                ps[:, c0:512],
                                Kt[m][off:off + DK, i * P:(i + 1) * P],
                                Qt[m][off:off + DK,
                                      jq * 512 + c0:(jq + 1) * 512],
                                start=True, stop=True)
                            return ps

                        prev = score(0)
                        for i in range(nk):
                            ps = prev
                            if i + 1 < nk:
                                prev = score(i + 1)
                            c0 = col0(i)
                            p_sb = ppool.tile([P, 512], BF16)
                            nc.scalar.activation(
                                p_sb[:, c0:512], ps[:, c0:512], AF.Exp,
                                scale=0.125)
                            if i >= 4 * jq:
                                # triangle-mask the 128-col diagonal band
                                nc.vector.tensor_mul(
                                    p_sb[:, c0:c0 + P],
                                    p_sb[:, c0:c0 + P], tri[:])
                            nc.tensor.matmul(
                                po[:, c0:512], V[:, i, h, :],
                                p_sb[:, c0:512],
                                start=(i == 0), stop=(i == nk - 1))

                        dsb = small.tile([1, 512], F32, tag="dsb")
                        nc.vector.tensor_copy(dsb[:], po[DK:DK + 1, :])
                        r = small.tile([1, 512], F32, tag="r")
                        nc.vector.reciprocal_approx_fast(r[:], dsb[:])
                        rb = small.tile([DK, 512], F32, tag="rb")
                        nc.gpsimd.partition_broadcast(rb[:], r[:], channels=DK)
                        # halves: wo t=0,1 start after the first one lands
                        for half in range(2):
                            cs = slice(half * 256, (half + 1) * 256)
                            nc.vector.tensor_mul(
                                Ot[m][off:off + DK, cs], po[0:DK, cs],
                                rb[:, cs])
                return Ot

            def wo_stage(jq, Ot):
                with nc.named_scope(f"wo{jq}"):
                    for t in range(4):
                        for n in range(2):
                            ps = psA.tile([P, 512], F32, tag="ps",
                                          name="ps")
                            nc.tensor.matmul(
                                ps[:], Ot[0][:, t * P:(t + 1) * P],
                                wo_sb[:, 0, n * 512:(n + 1) * 512],
                                start=True, stop=False)
                            nc.tensor.matmul(
                                ps[:], Ot[1][:, t * P:(t + 1) * P],
                                wo_sb[:, 1, n * 512:(n + 1) * 512],
                                start=False, stop=True)
                            osb = outsbp.tile([P, 512], BF16, tag="osb")
                            nc.vector.tensor_copy(osb[:], ps[:])
                            nc.sync.dma_start(
                                out=out[jq * 512 + t * P:
                                        jq * 512 + (t + 1) * P,
                                        n * 512:(n + 1) * 512],
                                in_=osb[:])

            xts = proj_dma(0)
            proj_qk(0, xts, "q")
            proj_qk(0, xts, "k")
            proj_v(0, xts)
            for j in range(NJ):
                Ot = att_heads(j)
                if j + 1 < NJ:
                    # Q-projection of the next chunk runs on the PE while
                    # the last head's normalization chain completes
                    xts = proj_dma(j + 1)
                    proj_qk(j + 1, xts, "q")
                wo_stage(j, Ot)
                if j + 1 < NJ:
                    proj_qk(j + 1, xts, "k")
                    proj_v(j + 1, xts)

    nc.compile()
    return nc


_NC_CACHE = None
_last_in_maps = None


def _get_nc():
    global _NC_CACHE
    if _NC_CACHE is None:
        _NC_CACHE = build_nc()
    return _NC_CACHE


def _reference_np(q, k, v, mask, wq, bq, wk, bk, wv, bv, wo, bo):
    """Plain numpy fallback (only used if mask is not causal)."""
    query = q @ wq.T + bq
    key_ = k @ wk.T + bk
    value = v @ wv.T + bv
    H = D // DK
    query = query.reshape(B, S, H, DK).transpose(0, 2, 1, 3)
    key_ = key_.reshape(B, S, H, DK).transpose(0, 2, 1, 3)
    value = value.reshape(B, S, H, DK).transpose(0, 2, 1, 3)
    scores = np.einsum("bhqd,bhkd->bhqk", query, key_) / np.sqrt(np.float32(DK))
    scores = np.where(mask == 0, np.float32(-1e9), scores)
    scores = scores - scores.max(axis=-1, keepdims=True)
    e = np.exp(scores)
    attn = e / e.sum(axis=-1, keepdims=True)
    x = np.einsum("bhqk,bhkd->bhqd", attn, value)
    x = x.transpose(0, 2, 1, 3).reshape(B, S, D)
    return (x @ wo.T + bo).astype(np.float32)


def kernel(q, k, v, mask, wq, bq, wk, bk, wv, bv, wo, bo, **_unused):
    q = np.asarray(q, np.float32)
    k = np.asarray(k, np.float32)
    v = np.asarray(v, np.float32)
    wq = np.asarray(wq, np.float32)
    wk = np.asarray(wk, np.float32)
    wv = np.asarray(wv, np.float32)
    wo = np.asarray(wo, np.float32)
    bq = np.asarray(bq, np.float32)
    bk = np.asarray(bk, np.float32)
    bv = np.asarray(bv, np.float32)
    bo = np.asarray(bo, np.float32)
    mask_np = np.asarray(mask)

    # the device kernel hardcodes causal masking; verify and fall back if not
    causal = np.tril(np.ones((S, S), np.int32))
    if not np.array_equal(mask_np.reshape(S, S).astype(np.int32), causal):
        return _reference_np(q, k, v, mask_np, wq, bq, wk, bk, wv, bv, wo, bo)

    nc = _get_nc()

    # host-side prep: transpose + cast once per batch / head-group
    xT = {}
    for b in range(B):
        xT[b] = {
            "xqT": np.ascontiguousarray(q[b].T).astype(NPBF),
            "xkT": np.ascontiguousarray(k[b].T).astype(NPBF),
            "xvT": np.ascontiguousarray(v[b].T).astype(NPBF),
        }
    wT = {}
    for g in range(4):
        sl = slice(g * HD, (g + 1) * HD)
        wT[g] = {
            "wqT": np.ascontiguousarray(wq[sl].T).astype(NPBF),
            "wkT": np.ascontiguousarray(wk[sl].T).astype(NPBF),
            "wvT": np.ascontiguousarray(wv[sl].T).astype(NPBF),
            "woT": np.ascontiguousarray(wo[:, sl].T).astype(NPBF),
            "bq": np.ascontiguousarray(bq[sl]).reshape(1, HD).astype(NPBF),
        }

    in_maps = []
    for c in range(NCORES):
        b, g = c // 4, c % 4
        in_maps.append({**xT[b], **wT[g]})

    global _last_in_maps
    _last_in_maps = in_maps
    res = run_bass_kernel_spmd(nc, in_maps, core_ids=list(range(NCORES)))

    # k-bias is softmax-invariant (dropped); v-bias passes through
    # attention unchanged, so it folds into the output bias exactly.
    bias = bo + wo @ bv
    out = np.empty((B, S, D), np.float32)
    for b in range(B):
        acc = res.results[4 * b]["out"].astype(np.float32).copy()
        for g in range(1, 4):
            acc += res.results[4 * b + g]["out"]
        out[b] = acc + bias[None, :]
    return out


# revision 18
# speedup vs baseline: 1.1620x; 1.1620x over previous
"""Trainium2 Bass kernel for nn_MultiHeadAttention_79224966742350.

Full (unsharded) inputs in, full output out. Internally: 8-way SPMD over
8 NeuronCores, sharded batch x head-group: core c handles batch c//4 and
heads [4*(c%4), 4*(c%4)+4) (=256 of the 1024 projection dims). Each core
computes its partial x @ wo_cols contribution; the host sums the 4
partials per batch and adds the (adjusted) output bias.

Key design:
  - The HOST pre-transposes and pre-casts everything the device would
    otherwise have to transpose on the PE: x^T [D, S] and w^T slices are
    shipped bf16, eliminating all PE transpose instructions.
  - All matmuls run in bf16 (1 cycle/row at any moving size on TRN2, no
    fp32r small-N penalty); PSUM accumulation stays fp32.
  - k-bias is dropped exactly (softmax shift invariance); v-bias is
    folded into the host-side output bias (bo' = bo + wo @ bv); q-bias
    is a K=1 ones matmul.
  - The next chunk's Q-projection is issued between attention and the
    output projection so the PE stays busy while the last head's
    normalization chain (DVE reciprocal + Pool broadcast) completes.
  - Engine assignment: Act = exp + Q/K evictions, DVE = V evictions /
    triangle mask / normalization / output evictions, Pool(GpSimd) =
    partition broadcast; PE does only real matmuls.

Device-side per core (S=2048 tokens, D=1024, 4 heads x d_k=64), per
512-token chunk j: project Q^T/K^T [head_dim, tok] and V [tok, head_dim]
from pre-transposed inputs, then attention for q-chunk j with transposed
scores S^T = K^T.T @ Q^T -> [k_tok, q_tok]: softmax numerator via ACT Exp
from PSUM restricted to the causally-live range, diagonal-band triangle
masking via DVE multiply, denominator via a ones column appended to V
(attnV matmul M=65), normalization via DVE reciprocal + Pool
partition_broadcast + DVE multiply, then the output projection.
"""

import sys

sys.path.insert(0, "/opt/trn_rl_repo")

import numpy as np
import ml_dtypes

import concourse.bacc as bacc
import concourse.mybir as mybir
import concourse.tile as tile
from concourse.bass_utils import run_bass_kernel_spmd

F32 = mybir.dt.float32
BF16 = mybir.dt.bfloat16
AF = mybir.ActivationFunctionType
NPBF = ml_dtypes.bfloat16

B = 2
S = 2048
D = 1024
DK = 64
HPC = 4          # heads per core
HD = HPC * DK    # 256 projection dims per core
NCORES = 8
NJ = S // 512    # 512-token chunks
P = 128


def build_nc():
    nc = bacc.Bacc("TRN2", target_bir_lowering=False, debug=False,
                   num_devices=NCORES)

    xqT = nc.dram_tensor("xqT", [D, S], BF16, kind="ExternalInput").ap()
    xkT = nc.dram_tensor("xkT", [D, S], BF16, kind="ExternalInput").ap()
    xvT = nc.dram_tensor("xvT", [D, S], BF16, kind="ExternalInput").ap()
    wqT = nc.dram_tensor("wqT", [D, HD], BF16, kind="ExternalInput").ap()
    wkT = nc.dram_tensor("wkT", [D, HD], BF16, kind="ExternalInput").ap()
    wvT = nc.dram_tensor("wvT", [D, HD], BF16, kind="ExternalInput").ap()
    woT = nc.dram_tensor("woT", [HD, D], BF16, kind="ExternalInput").ap()
    bq = nc.dram_tensor("bq", [1, HD], BF16, kind="ExternalInput").ap()
    out = nc.dram_tensor("out", [S, D], BF16, kind="ExternalOutput").ap()

    with tile.TileContext(nc) as tc:
        with (
            tc.tile_pool(name="const", bufs=1) as const,
            tc.tile_pool(name="wtp", bufs=1) as wtp,
            tc.tile_pool(name="qkv", bufs=1) as qkv,
            tc.tile_pool(name="opool", bufs=2) as opool,
            tc.tile_pool(name="xtp", bufs=2) as xtp,
            tc.tile_pool(name="ppool", bufs=6) as ppool,
            tc.tile_pool(name="small", bufs=2) as small,
            tc.tile_pool(name="outsb", bufs=3) as outsbp,
            tc.tile_pool(name="psA", bufs=2, space="PSUM") as psA,
            tc.tile_pool(name="psS", bufs=3, space="PSUM") as psS,
            tc.tile_pool(name="psO", bufs=2, space="PSUM") as psO,
            tc.tile_pool(name="psW", bufs=1, space="PSUM") as psW,
        ):
            ones = const.tile([1, 512], BF16, tag="ones")
            nc.gpsimd.memset(ones[:], 1.0)
            # triangle mask: tri[p, f] = 1.0 where f >= p else 0.0
            tri = const.tile([P, P], BF16, tag="tri")
            nc.gpsimd.memset(tri[:], 1.0)
            nc.gpsimd.affine_select(
                out=tri[:], in_=tri[:],
                compare_op=mybir.AluOpType.is_ge, fill=0.0,
                base=0, pattern=[[1, P]], channel_multiplier=-1)

            # exp-table preload: a 1-elem Exp so att0's first real exp
            # doesn't pay the ACT_TABLE_LOAD
            dummy = const.tile([1, 1], F32, tag="dummy")
            nc.scalar.activation(dummy[:], dummy[:], AF.Exp)

            # ---- weights: DMA pre-transposed bf16 straight into SBUF ----
            # issue order tracks first use: wq/xq gate the first matmul,
            # wo_sb isn't needed until wo0
            wq_sb = wtp.tile([P, 8, HD], BF16, tag="wq")
            wk_sb = wtp.tile([P, 8, HD], BF16, tag="wk")
            wv_sb = wtp.tile([P, 8, HD], BF16, tag="wv")
            wo_sb = wtp.tile([P, 2, D], BF16, tag="wo")
            bq_sb = wtp.tile([1, HD], BF16, tag="bq")
            wqr = wqT.rearrange("(d p) c -> p d c", p=P)
            for dd in range(8):
                nc.sync.dma_start(out=wq_sb[:, dd, :], in_=wqr[:, dd, :])
            nc.sync.dma_start(out=bq_sb[:], in_=bq)

            # ---- storage for Q^T, K^T (transposed) and V (natural) ------
            Qt = [qkv.tile([P, S], BF16, tag=f"Q{m}", name=f"Q{m}")
                  for m in range(2)]
            Kt = [qkv.tile([P, S], BF16, tag=f"K{m}", name=f"K{m}")
                  for m in range(2)]
            # V with a ones column per head: [tok_tile, tok, head, 65]
            V = qkv.tile([P, 16, HPC, DK + 1], BF16, tag="V")
            nc.gpsimd.memset(V[:, :, :, DK:DK + 1], 1.0)

            # ---- projections for one 512-token chunk --------------------
            def proj_dma(j):
                xts = {}
                for name, x_ap in (("q", xqT), ("k", xkT), ("v", xvT)):
                    xT = xtp.tile([P, 8, 512], BF16, tag=f"xT{name}",
                                  name=f"xT{name}")
                    xr = x_ap.rearrange("(d p) s -> p d s", p=P)
                    if j == 0:
                        # d-sliced so the first matmuls gate on ~192KB
                        # instead of the full 1.5MB
                        for dd in range(8):
                            nc.sync.dma_start(
                                out=xT[:, dd, :],
                                in_=xr[:, dd, j * 512:(j + 1) * 512])
                    else:
                        nc.sync.dma_start(
                            out=xT[:], in_=xr[:, :, j * 512:(j + 1) * 512])
                    xts[name] = xT
                    if j == 0 and name == "q":
                        nc.sync.dma_start(
                            out=wk_sb[:],
                            in_=wkT.rearrange("(d p) c -> p d c", p=P))
                    elif j == 0 and name == "k":
                        nc.sync.dma_start(
                            out=wv_sb[:],
                            in_=wvT.rearrange("(d p) c -> p d c", p=P))
                    elif j == 0 and name == "v":
                        nc.sync.dma_start(
                            out=wo_sb[:],
                            in_=woT.rearrange("(m p) c -> p m c", p=P))
                return xts

            def proj_qk(j, xts, which):
                # Q^T or K^T: [head_dim 256, tok 512] for this chunk
                with nc.named_scope(f"proj{j}{which}"):
                    xT, wsb, b_sb, dst = {
                        "q": (xts["q"], wq_sb, bq_sb, Qt),
                        "k": (xts["k"], wk_sb, None, Kt),
                    }[which]
                    for m in range(2):
                        ps = psA.tile([P, 512], F32, tag="ps", name="ps")
                        for d in range(8):
                            nc.tensor.matmul(
                                ps[:], wsb[:, d, m * P:(m + 1) * P],
                                xT[:, d, :], start=(d == 0),
                                stop=(d == 7 and b_sb is None))
                        if b_sb is not None:
                            nc.tensor.matmul(
                                ps[:], b_sb[0:1, m * P:(m + 1) * P],
                                ones[:], start=False, stop=True)
                        nc.scalar.activation(
                            dst[m][:, j * 512:(j + 1) * 512], ps[:], AF.Copy)

            def proj_v(j, xts):
                # V natural: [tok 128, head_dim 256] per token tile
                with nc.named_scope(f"proj{j}v"):
                    for t in range(4):
                        ps = psA.tile([P, 512], F32, tag="ps", name="ps")
                        for d in range(8):
                            nc.tensor.matmul(
                                ps[:, 0:HD], xts["v"][:, d, t * P:(t + 1) * P],
                                wv_sb[:, d, :], start=(d == 0), stop=(d == 7))
                        nc.vector.tensor_copy(
                            V[:, j * 4 + t, :, 0:DK],
                            ps[:, 0:HD].rearrange("p (h c) -> p h c", c=DK))

            # ---- attention + output projection for one 512-q chunk ------
            def att_heads(jq):
                Ot = [opool.tile([P, 512], BF16, tag=f"O{m}", name=f"O{m}")
                      for m in range(2)]
                with nc.named_scope(f"att{jq}"):
                    for h in range(HPC):
                        m, off = h // 2, DK * (h % 2)
                        nk = 4 * (jq + 1)
                        po = psO.tile([DK + 1, 512], F32)

                        def col0(i):
                            # first causally-live column of k-chunk i's tile
                            return max(0, 128 * i - 512 * jq)

                        def score(i):
                            c0 = col0(i)
                            ps = psS.tile([P, 512], F32)
                            nc.tensor.matmul(
                                ps[:, c0:512],
                                Kt[m][off:off + DK, i * P:(i + 1) * P],
                                Qt[m][off:off + DK,
                                      jq * 512 + c0:(jq + 1) * 512],
                                start=True, stop=True)
                            return ps

                        prev = score(0)
                        for i in range(nk):
                            ps = prev
                            if i + 1 < nk:
                                prev = score(i + 1)
                            c0 = col0(i)
                            p_sb = ppool.tile([P, 512], BF16)
                            nc.scalar.activation(
                                p_sb[:, c0:512], ps[:, c0:512], AF.Exp,
                                scale=0.125)
                            if i >= 4 * jq:
                                # triangle-mask the 128-col diagonal band
                                nc.vector.tensor_mul(
                                    p_sb[:, c0:c0 + P],
                                    p_sb[:, c0:c0 + P], tri[:])
                            nc.tensor.matmul(
                                po[:, c0:512], V[:, i, h, :],
                                p_sb[:, c0:512],
                                start=(i == 0), stop=(i == nk - 1))

                        dsb = small.tile([1, 512], F32, tag="dsb")
                        nc.vector.tensor_copy(dsb[:], po[DK:DK + 1, :])
                        r = small.tile([1, 512], F32, tag="r")
                        nc.vector.reciprocal_approx_fast(r[:], dsb[:])
                        rb = small.tile([DK, 512], F32, tag="rb")
                        nc.gpsimd.partition_broadcast(rb[:], r[:], channels=DK)
                        # halves: wo t=0,1 start after the first one lands
                        for half in range(2):
                            cs = slice(half * 256, (half + 1) * 256)
                            nc.vector.tensor_mul(
                                Ot[m][off:off + DK, cs], po[0:DK, cs],
                                rb[:, cs])
                return Ot

            def wo_stage(jq, Ot):
                with nc.named_scope(f"wo{jq}"):
                    for t in range(4):
                        for n in range(2):
                            ps = psW.tile([P, 512], F32, name="ps")
                            nc.tensor.matmul(
                                ps[:], Ot[0][:, t * P:(t + 1) * P],
                                wo_sb[:, 0, n * 512:(n + 1) * 512],
                                start=True, stop=False)
                            nc.tensor.matmul(
                                ps[:], Ot[1][:, t * P:(t + 1) * P],
                                wo_sb[:, 1, n * 512:(n + 1) * 512],
                                start=False, stop=True)
                            osb = outsbp.tile([P, 512], BF16, tag="osb")
                            nc.vector.tensor_copy(osb[:], ps[:])
                            nc.sync.dma_start(
                                out=out[jq * 512 + t * P:
                                        jq * 512 + (t + 1) * P,
                                        n * 512:(n + 1) * 512],
                                in_=osb[:])

            xts = proj_dma(0)
            proj_qk(0, xts, "q")
            proj_qk(0, xts, "k")
            proj_v(0, xts)
            for j in range(NJ):
                Ot = att_heads(j)
                if j + 1 < NJ:
                    # Q-projection of the next chunk runs on the PE while
                    # the last head's normalization chain completes
                    xts = proj_dma(j + 1)
                    proj_qk(j + 1, xts, "q")
                wo_stage(j, Ot)
                if j + 1 < NJ:
                    proj_qk(j + 1, xts, "k")
                    proj_v(j + 1, xts)

    nc.compile()
    return nc


_NC_CACHE = None
_last_in_maps = None


def _get_nc():
    global _NC_CACHE
    if _NC_CACHE is None:
        _NC_CACHE = build_nc()
    return _NC_CACHE


def _reference_np(q, k, v, mask, wq, bq, wk, bk, wv, bv, wo, bo):
    """Plain numpy fallback (only used if mask is not causal)."""
    query = q @ wq.T + bq
    key_ = k @ wk.T + bk
    value = v @ wv.T + bv
    H = D // DK
    query = query.reshape(B, S, H, DK).transpose(0, 2, 1, 3)
    key_ = key_.reshape(B, S, H, DK).transpose(0, 2, 1, 3)
    value = value.reshape(B, S, H, DK).transpose(0, 2, 1, 3)
    scores = np.einsum("bhqd,bhkd->bhqk", query, key_) / np.sqrt(np.float32(DK))
    scores = np.where(mask == 0, np.float32(-1e9), scores)
    scores = scores - scores.max(axis=-1, keepdims=True)
    e = np.exp(scores)
    attn = e / e.sum(axis=-1, keepdims=True)
    x = np.einsum("bhqk,bhkd->bhqd", attn, value)
    x = x.transpose(0, 2, 1, 3).reshape(B, S, D)
    return (x @ wo.T + bo).astype(np.float32)


def kernel(q, k, v, mask, wq, bq, wk, bk, wv, bv, wo, bo, **_unused):
    q = np.asarray(q, np.float32)
    k = np.asarray(k, np.float32)
    v = np.asarray(v, np.float32)
    wq = np.asarray(wq, np.float32)
    wk = np.asarray(wk, np.float32)
    wv = np.asarray(wv, np.float32)
    wo = np.asarray(wo, np.float32)
    bq = np.asarray(bq, np.float32)
    bk = np.asarray(bk, np.float32)
    bv = np.asarray(bv, np.float32)
    bo = np.asarray(bo, np.float32)
    mask_np = np.asarray(mask)

    # the device kernel hardcodes causal masking; verify and fall back if not
    causal = np.tril(np.ones((S, S), np.int32))
    if not np.array_equal(mask_np.reshape(S, S).astype(np.int32), causal):
        return _reference_np(q, k, v, mask_np, wq, bq, wk, bk, wv, bv, wo, bo)

    nc = _get_nc()

    # host-side prep: transpose + cast once per batch / head-group
    xT = {}
    for b in range(B):
        xT[b] = {
            "xqT": np.ascontiguousarray(q[b].T).astype(NPBF),
            "xkT": np.ascontiguousarray(k[b].T).astype(NPBF),
            "xvT": np.ascontiguousarray(v[b].T).astype(NPBF),
        }
    wT = {}
    for g in range(4):
        sl = slice(g * HD, (g + 1) * HD)
        wT[g] = {
            "wqT": np.ascontiguousarray(wq[sl].T).astype(NPBF),
            "wkT": np.ascontiguousarray(wk[sl].T).astype(NPBF),
            "wvT": np.ascontiguousarray(wv[sl].T).astype(NPBF),
            "woT": np.ascontiguousarray(wo[:, sl].T).astype(NPBF),
            "bq": np.ascontiguousarray(bq[sl]).reshape(1, HD).astype(NPBF),
        }

    in_maps = []
    for c in range(NCORES):
        b, g = c // 4, c % 4
        in_maps.append({**xT[b], **wT[g]})

    global _last_in_maps
    _last_in_maps = in_maps
    res = run_bass_kernel_spmd(nc, in_maps, core_ids=list(range(NCORES)))

    # k-bias is softmax-invariant (dropped); v-bias passes through
    # attention unchanged, so it folds into the output bias exactly.
    bias = bo + wo @ bv
    out = np.empty((B, S, D), np.float32)
    for b in range(B):
        acc = res.results[4 * b]["out"].astype(np.float32).copy()
        for g in range(1, 4):
            acc += res.results[4 * b + g]["out"]
        out[b] = acc + bias[None, :]
    return out


# revision 19
# speedup vs baseline: 1.1966x; 1.0298x over previous
"""Trainium2 Bass kernel for nn_MultiHeadAttention_79224966742350.

Full (unsharded) inputs in, full output out. Internally: 8-way SPMD over
8 NeuronCores, sharded batch x head-group: core c handles batch c//4 and
heads [4*(c%4), 4*(c%4)+4) (=256 of the 1024 projection dims). Each core
computes its partial x @ wo_cols contribution; the host sums the 4
partials per batch and adds the (adjusted) output bias.

Key design:
  - The HOST pre-transposes and pre-casts everything the device would
    otherwise have to transpose on the PE: x^T [D, S] and w^T slices are
    shipped bf16, eliminating all PE transpose instructions.
  - All matmuls run in bf16 (1 cycle/row at any moving size on TRN2, no
    fp32r small-N penalty); PSUM accumulation stays fp32.
  - k-bias is dropped exactly (softmax shift invariance); v-bias is
    folded into the host-side output bias (bo' = bo + wo @ bv); q-bias
    is a K=1 ones matmul.
  - The next chunk's Q-projection is issued between attention and the
    output projection so the PE stays busy while the last head's
    normalization chain (DVE reciprocal + Pool broadcast) completes.
  - Engine assignment: Act = exp + Q/K evictions, DVE = V evictions /
    triangle mask / normalization / output evictions, Pool(GpSimd) =
    partition broadcast; PE does only real matmuls.

Device-side per core (S=2048 tokens, D=1024, 4 heads x d_k=64), per
512-token chunk j: project Q^T/K^T [head_dim, tok] and V [tok, head_dim]
from pre-transposed inputs, then attention for q-chunk j with transposed
scores S^T = K^T.T @ Q^T -> [k_tok, q_tok]: softmax numerator via ACT Exp
from PSUM restricted to the causally-live range, diagonal-band triangle
masking via DVE multiply, denominator via a ones column appended to V
(attnV matmul M=65), normalization via DVE reciprocal + Pool
partition_broadcast + DVE multiply, then the output projection.
"""

import sys

sys.path.insert(0, "/opt/trn_rl_repo")

import numpy as np
import ml_dtypes

import concourse.bacc as bacc
import concourse.mybir as mybir
import concourse.tile as tile
from concourse.bass_utils import run_bass_kernel_spmd

F32 = mybir.dt.float32
BF16 = mybir.dt.bfloat16
AF = mybir.ActivationFunctionType
NPBF = ml_dtypes.bfloat16

B = 2
S = 2048
D = 1024
DK = 64
HPC = 4          # heads per core
HD = HPC * DK    # 256 projection dims per core
NCORES = 8
NJ = S // 512    # 512-token chunks
P = 128


def build_nc():
    nc = bacc.Bacc("TRN2", target_bir_lowering=False, debug=False,
                   num_devices=NCORES)

    xqT = nc.dram_tensor("xqT", [D, S], BF16, kind="ExternalInput").ap()
    xkT = nc.dram_tensor("xkT", [D, S], BF16, kind="ExternalInput").ap()
    xvT = nc.dram_tensor("xvT", [D, S], BF16, kind="ExternalInput").ap()
    wqT = nc.dram_tensor("wqT", [D, HD], BF16, kind="ExternalInput").ap()
    wkT = nc.dram_tensor("wkT", [D, HD], BF16, kind="ExternalInput").ap()
    wvT = nc.dram_tensor("wvT", [D, HD], BF16, kind="ExternalInput").ap()
    woT = nc.dram_tensor("woT", [HD, D], BF16, kind="ExternalInput").ap()
    bq = nc.dram_tensor("bq", [1, HD], BF16, kind="ExternalInput").ap()
    out = nc.dram_tensor("out", [S, D], BF16, kind="ExternalOutput").ap()

    with tile.TileContext(nc) as tc:
        with (
            tc.tile_pool(name="const", bufs=1) as const,
            tc.tile_pool(name="wtp", bufs=1) as wtp,
            tc.tile_pool(name="qkv", bufs=1) as qkv,
            tc.tile_pool(name="opool", bufs=2) as opool,
            tc.tile_pool(name="xtp", bufs=2) as xtp,
            tc.tile_pool(name="ppool", bufs=6) as ppool,
            tc.tile_pool(name="small", bufs=2) as small,
            tc.tile_pool(name="outsb", bufs=3) as outsbp,
            tc.tile_pool(name="psA", bufs=2, space="PSUM") as psA,
            tc.tile_pool(name="psS", bufs=3, space="PSUM") as psS,
            tc.tile_pool(name="psO", bufs=2, space="PSUM") as psO,
            tc.tile_pool(name="psW", bufs=1, space="PSUM") as psW,
        ):
            ones = const.tile([1, 512], BF16, tag="ones")
            nc.gpsimd.memset(ones[:], 1.0)
            # triangle mask: tri[p, f] = 1.0 where f >= p else 0.0
            tri = const.tile([P, P], BF16, tag="tri")
            nc.gpsimd.memset(tri[:], 1.0)
            nc.gpsimd.affine_select(
                out=tri[:], in_=tri[:],
                compare_op=mybir.AluOpType.is_ge, fill=0.0,
                base=0, pattern=[[1, P]], channel_multiplier=-1)

            # exp-table preload: a 1-elem Exp so att0's first real exp
            # doesn't pay the ACT_TABLE_LOAD
            dummy = const.tile([1, 1], F32, tag="dummy")
            nc.scalar.activation(dummy[:], dummy[:], AF.Exp)

            # ---- weights: DMA pre-transposed bf16 straight into SBUF ----
            # issue order tracks first use: wq/xq gate the first matmul,
            # wo_sb isn't needed until wo0
            wq_sb = wtp.tile([P, 8, HD], BF16, tag="wq")
            wk_sb = wtp.tile([P, 8, HD], BF16, tag="wk")
            wv_sb = wtp.tile([P, 8, HD], BF16, tag="wv")
            wo_sb = wtp.tile([P, 2, D], BF16, tag="wo")
            bq_sb = wtp.tile([1, HD], BF16, tag="bq")
            nc.sync.dma_start(
                out=wq_sb[:], in_=wqT.rearrange("(d p) c -> p d c", p=P))
            nc.sync.dma_start(out=bq_sb[:], in_=bq)

            # ---- storage for Q^T, K^T (transposed) and V (natural) ------
            Qt = [qkv.tile([P, S], BF16, tag=f"Q{m}", name=f"Q{m}")
                  for m in range(2)]
            Kt = [qkv.tile([P, S], BF16, tag=f"K{m}", name=f"K{m}")
                  for m in range(2)]
            # V with a ones column per head: [tok_tile, tok, head, 65]
            V = qkv.tile([P, 16, HPC, DK + 1], BF16, tag="V")
            nc.gpsimd.memset(V[:, :, :, DK:DK + 1], 1.0)

            # ---- projections for one 512-token chunk --------------------
            def proj_dma(j):
                xts = {}
                for name, x_ap in (("q", xqT), ("k", xkT), ("v", xvT)):
                    xT = xtp.tile([P, 8, 512], BF16, tag=f"xT{name}",
                                  name=f"xT{name}")
                    xr = x_ap.rearrange("(d p) s -> p d s", p=P)
                    nc.sync.dma_start(
                        out=xT[:], in_=xr[:, :, j * 512:(j + 1) * 512])
                    xts[name] = xT
                    if j == 0 and name == "q":
                        nc.sync.dma_start(
                            out=wk_sb[:],
                            in_=wkT.rearrange("(d p) c -> p d c", p=P))
                    elif j == 0 and name == "k":
                        nc.sync.dma_start(
                            out=wv_sb[:],
                            in_=wvT.rearrange("(d p) c -> p d c", p=P))
                    elif j == 0 and name == "v":
                        nc.sync.dma_start(
                            out=wo_sb[:],
                            in_=woT.rearrange("(m p) c -> p m c", p=P))
                return xts

            def proj_qk(j, xts, which):
                # Q^T or K^T: [head_dim 256, tok 512] for this chunk
                with nc.named_scope(f"proj{j}{which}"):
                    xT, wsb, b_sb, dst = {
                        "q": (xts["q"], wq_sb, bq_sb, Qt),
                        "k": (xts["k"], wk_sb, None, Kt),
                    }[which]
                    for m in range(2):
                        ps = psA.tile([P, 512], F32, tag="ps", name="ps")
                        for d in range(8):
                            nc.tensor.matmul(
                                ps[:], wsb[:, d, m * P:(m + 1) * P],
                                xT[:, d, :], start=(d == 0),
                                stop=(d == 7 and b_sb is None))
                        if b_sb is not None:
                            nc.tensor.matmul(
                                ps[:], b_sb[0:1, m * P:(m + 1) * P],
                                ones[:], start=False, stop=True)
                        nc.scalar.activation(
                            dst[m][:, j * 512:(j + 1) * 512], ps[:], AF.Copy)

            def proj_v(j, xts):
                # V natural: [tok 128, head_dim 256] per token tile
                with nc.named_scope(f"proj{j}v"):
                    for t in range(4):
                        ps = psA.tile([P, 512], F32, tag="ps", name="ps")
                        for d in range(8):
                            nc.tensor.matmul(
                                ps[:, 0:HD], xts["v"][:, d, t * P:(t + 1) * P],
                                wv_sb[:, d, :], start=(d == 0), stop=(d == 7))
                        nc.vector.tensor_copy(
                            V[:, j * 4 + t, :, 0:DK],
                            ps[:, 0:HD].rearrange("p (h c) -> p h c", c=DK))

            # ---- attention + output projection for one 512-q chunk ------
            def att_heads(jq):
                Ot = [opool.tile([P, 512], BF16, tag=f"O{m}", name=f"O{m}")
                      for m in range(2)]
                with nc.named_scope(f"att{jq}"):
                    for h in range(HPC):
                        m, off = h // 2, DK * (h % 2)
                        nk = 4 * (jq + 1)
                        po = psO.tile([DK + 1, 512], F32)

                        def col0(i):
                            # first causally-live column of k-chunk i's tile
                            return max(0, 128 * i - 512 * jq)

                        def score(i):
                            c0 = col0(i)
                            ps = psS.tile([P, 512], F32)
                            nc.tensor.matmul(
                                ps[:, c0:512],
                                Kt[m][off:off + DK, i * P:(i + 1) * P],
                                Qt[m][off:off + DK,
                                      jq * 512 + c0:(jq + 1) * 512],
                                start=True, stop=True)
                            return ps

                        prev = score(0)
                        for i in range(nk):
                            ps = prev
                            if i + 1 < nk:
                                prev = score(i + 1)
                            c0 = col0(i)
                            p_sb = ppool.tile([P, 512], BF16)
                            nc.scalar.activation(
                                p_sb[:, c0:512], ps[:, c0:512], AF.Exp,
                                scale=0.125)
                            if i >= 4 * jq:
                                # triangle-mask the 128-col diagonal band
                                nc.vector.tensor_mul(
                                    p_sb[:, c0:c0 + P],
                                    p_sb[:, c0:c0 + P], tri[:])
                            nc.tensor.matmul(
                                po[:, c0:512], V[:, i, h, :],
                                p_sb[:, c0:512],
                                start=(i == 0), stop=(i == nk - 1))

                        dsb = small.tile([1, 512], F32, tag="dsb")
                        nc.vector.tensor_copy(dsb[:], po[DK:DK + 1, :])
                        r = small.tile([1, 512], F32, tag="r")
                        nc.vector.reciprocal_approx_fast(r[:], dsb[:])
                        rb = small.tile([DK, 512], F32, tag="rb")
                        nc.gpsimd.partition_broadcast(rb[:], r[:], channels=DK)
                        # halves: wo t=0,1 start after the first one lands
                        for half in range(2):
                            cs = slice(half * 256, (half + 1) * 256)
                            nc.vector.tensor_mul(
                                Ot[m][off:off + DK, cs], po[0:DK, cs],
                                rb[:, cs])
                return Ot

            def wo_stage(jq, Ot):
                with nc.named_scope(f"wo{jq}"):
                    for t in range(4):
                        for n in range(2):
                            ps = psW.tile([P, 512], F32, name="ps")
                            nc.tensor.matmul(
                                ps[:], Ot[0][:, t * P:(t + 1) * P],
                                wo_sb[:, 0, n * 512:(n + 1) * 512],
                                start=True, stop=False)
                            nc.tensor.matmul(
                                ps[:], Ot[1][:, t * P:(t + 1) * P],
                                wo_sb[:, 1, n * 512:(n + 1) * 512],
                                start=False, stop=True)
                            osb = outsbp.tile([P, 512], BF16, tag="osb")
                            nc.vector.tensor_copy(osb[:], ps[:])
                            nc.sync.dma_start(
                                out=out[jq * 512 + t * P:
                                        jq * 512 + (t + 1) * P,
                                        n * 512:(n + 1) * 512],
                                in_=osb[:])

            xts = proj_dma(0)
            proj_qk(0, xts, "q")
            proj_qk(0, xts, "k")
            proj_v(0, xts)
            for j in range(NJ):
                Ot = att_heads(j)
                if j + 1 < NJ:
                    # Q-projection of the next chunk runs on the PE while
                    # the last head's normalization chain completes
                    xts = proj_dma(j + 1)
                    proj_qk(j + 1, xts, "q")
                wo_stage(j, Ot)
                if j + 1 < NJ:
                    proj_qk(j + 1, xts, "k")
                    proj_v(j + 1, xts)

    nc.compile()
    return nc


_NC_CACHE = None
_last_in_maps = None


def _get_nc():
    global _NC_CACHE
    if _NC_CACHE is None:
        _NC_CACHE = build_nc()
    return _NC_CACHE


def _reference_np(q, k, v, mask, wq, bq, wk, bk, wv, bv, wo, bo):
    """Plain numpy fallback (only used if mask is not causal)."""
    query = q @ wq.T + bq
    key_ = k @ wk.T + bk
    value = v @ wv.T + bv
    H = D // DK
    query = query.reshape(B, S, H, DK).transpose(0, 2, 1, 3)
    key_ = key_.reshape(B, S, H, DK).transpose(0, 2, 1, 3)
    value = value.reshape(B, S, H, DK).transpose(0, 2, 1, 3)
    scores = np.einsum("bhqd,bhkd->bhqk", query, key_) / np.sqrt(np.float32(DK))
    scores = np.where(mask == 0, np.float32(-1e9), scores)
    scores = scores - scores.max(axis=-1, keepdims=True)
    e = np.exp(scores)
    attn = e / e.sum(axis=-1, keepdims=True)
    x = np.einsum("bhqk,bhkd->bhqd", attn, value)
    x = x.transpose(0, 2, 1, 3).reshape(B, S, D)
    return (x @ wo.T + bo).astype(np.float32)


def kernel(q, k, v, mask, wq, bq, wk, bk, wv, bv, wo, bo, **_unused):
    q = np.asarray(q, np.float32)
    k = np.asarray(k, np.float32)
    v = np.asarray(v, np.float32)
    wq = np.asarray(wq, np.float32)
    wk = np.asarray(wk, np.float32)
    wv = np.asarray(wv, np.float32)
    wo = np.asarray(wo, np.float32)
    bq = np.asarray(bq, np.float32)
    bk = np.asarray(bk, np.float32)
    bv = np.asarray(bv, np.float32)
    bo = np.asarray(bo, np.float32)
    mask_np = np.asarray(mask)

    # the device kernel hardcodes causal masking; verify and fall back if not
    causal = np.tril(np.ones((S, S), np.int32))
    if not np.array_equal(mask_np.reshape(S, S).astype(np.int32), causal):
        return _reference_np(q, k, v, mask_np, wq, bq, wk, bk, wv, bv, wo, bo)

    nc = _get_nc()

    # host-side prep: transpose + cast once per batch / head-group
    xT = {}
    for b in range(B):
        xT[b] = {
            "xqT": np.ascontiguousarray(q[b].T).astype(NPBF),
            "xkT": np.ascontiguousarray(k[b].T).astype(NPBF),
            "xvT": np.ascontiguousarray(v[b].T).astype(NPBF),
        }
    wT = {}
    for g in range(4):
        sl = slice(g * HD, (g + 1) * HD)
        wT[g] = {
            "wqT": np.ascontiguousarray(wq[sl].T).astype(NPBF),
            "wkT": np.ascontiguousarray(wk[sl].T).astype(NPBF),
            "wvT": np.ascontiguousarray(wv[sl].T).astype(NPBF),
            "woT": np.ascontiguousarray(wo[:, sl].T).astype(NPBF),
            "bq": np.ascontiguousarray(bq[sl]).reshape(1, HD).astype(NPBF),
        }

    in_maps = []
    for c in range(NCORES):
        b, g = c // 4, c % 4
        in_maps.append({**xT[b], **wT[g]})

    global _last_in_maps
    _last_in_maps = in_maps
    res = run_bass_kernel_spmd(nc, in_maps, core_ids=list(range(NCORES)))

    # k-bias is softmax-invariant (dropped); v-bias passes through
    # attention unchanged, so it folds into the output bias exactly.
    bias = bo + wo @ bv
    out = np.empty((B, S, D), np.float32)
    for b in range(B):
        acc = res.results[4 * b]["out"].astype(np.float32).copy()
        for g in range(1, 4):
            acc += res.results[4 * b + g]["out"]
        out[b] = acc + bias[None, :]
    return out
